# revision 112
# baseline (speedup 1.0000x reference)
"""NeRF-style render kernel for TRN2 (8 NeuronCores, data-parallel over rays).

Self-contained: hardcodes all shapes. Coarse proposal MLP runs in fp32
(resampling is precision-critical), fine MLP in float32r.
"""
import os
import sys

sys.path.insert(0, '/opt/trn_rl_repo')
import numpy as np
import concourse.bass as bass
import concourse.bacc as bacc
import concourse.tile as tile
import concourse.mybir as mybir
from concourse.bass_utils import run_bass_kernel_spmd

F32 = mybir.dt.float32
F32R = mybir.dt.float32r
AF = mybir.ActivationFunctionType
OP = mybir.AluOpType

NCORES = 8
R = 128          # rays per core
S = 128          # samples per pass
CHUNK_RAYS = 16  # rays per chunk
NCHUNK = R // CHUNK_RAYS          # 8
CN = CHUNK_RAYS * S               # 2048 cols per chunk
TILE_N = 512                      # matmul moving size
NTILE = CN // TILE_N              # 4 point-tiles per chunk

MAGIC = np.float32(12582912.0)    # 1.5 * 2^23 (round-to-int trick)
INV2PI = np.float32(1.0 / (2.0 * np.pi))
C1 = np.float32(6.28125)          # 2*pi split, k*C1 exact for k < 2^13
C2 = np.float32(2.0 * np.pi - 6.28125)

BUILD_STAGE = int(os.environ.get("KERNEL_STAGE", "3"))
DEBUG_OUT = os.environ.get("KERNEL_DEBUG", "0") == "1"


# ---------------------------------------------------------------- host prep
def _posenc_rows(nf, span=None, minp=None):
    """A3 [6*nf,3] / const [6*nf] for rows f-major: per f: 3 sin, 3 cos."""
    rows = 6 * nf
    A3 = np.zeros((rows, 3), np.float64)
    ph = np.zeros((rows,), np.float64)
    for f in range(nf):
        for k in range(6):
            r = 6 * f + k
            d = k % 3
            sc = 2.0 ** f
            if span is not None:
                A3[r, d] = sc / span[d]
                ph[r] = -sc * minp[d] / span[d]
            else:
                A3[r, d] = sc
            if k >= 3:
                ph[r] += np.pi / 2.0
    return A3, ph


def host_prep(inp):
    c = {}
    f32 = np.float32

    # coarse posenc: selector*2^f matrix [3,60] + phase col [60,1]
    A3s, phs = _posenc_rows(10)
    c['cA3selT'] = np.concatenate(
        [A3s.T, np.zeros((3, 4))], 1).astype(f32)                # [3,64]
    c['cphasecol'] = np.concatenate(
        [phs, np.zeros(4)]).astype(f32).reshape(-1, 1)           # [64,1]

    # fine posenc rows: [sinx60, sinapp36, xyz3, appx3]
    minp = inp['min_point'].astype(np.float64)
    span = (inp['max_point'] - inp['min_point']).astype(np.float64)
    A3a, pha = _posenc_rows(6, span=span, minp=minp)
    pad4 = np.zeros((4, 3))
    fA3 = np.concatenate([A3s, pad4, A3a, np.eye(3), np.diag(1.0 / span)], 0)
    fph = np.concatenate([phs, np.zeros(4), pha, np.zeros(3), -minp / span], 0)
    c['fA3T'] = fA3.T.astype(f32).copy()                         # [3,106]
    c['fA4T'] = np.concatenate([fA3, fph[:, None]], 1).T.astype(f32).copy()

    # per-ray enc matrices (lhsT)
    Ad = np.zeros((24, 4), np.float64)
    for f in range(4):
        for k in range(6):
            r = 6 * f + k
            Ad[r, k % 3] = 2.0 ** f
            if k >= 3:
                Ad[r, 3] = np.pi / 2.0
    _AdT = Ad.T.astype(f32)
    At = np.zeros((12, 2), np.float64)
    for f in range(6):
        At[2 * f, 0] = 2.0 ** f
        At[2 * f + 1, 0] = 2.0 ** f
        At[2 * f + 1, 1] = np.pi / 2.0
    geo = np.zeros((4, 312), f32)
    geo[0:4, 0:24] = _AdT
    geo[0:2, 24:36] = At.T
    geo[0:3, 36:142] = c.pop('fA3T')
    geo[0:4, 142:248] = c.pop('fA4T')
    geo[0:3, 248:312] = c.pop('cA3selT')
    c['geopack'] = geo

    perm63 = list(range(3, 63)) + [0, 1, 2]
    c['pW0my67'] = np.concatenate(
        [inp['pW0'][3:63], np.zeros((4, 128), f32),
         inp['pW0'][0:3]], 0).astype(f32)                        # [67,128]
    c['pwpack'] = np.concatenate(
        [inp['pW1'], inp['pW2'], inp['pWo']], 1).astype(f32)     # [128,257]
    c['pb0col'] = inp['pb0'].reshape(-1, 1).copy()
    c['pb1col'] = inp['pb1'].reshape(-1, 1).copy()
    c['pb2col'] = inp['pb2'].reshape(-1, 1).copy()


    def pack_km(Wm):  # [256, 256] -> [128, 4, 128], slot 2k+m
        out = np.zeros((128, 4, 128), f32)
        for k in range(2):
            for m in range(2):
                out[:, 2 * k + m, :] = Wm[k * 128:(k + 1) * 128,
                                          m * 128:(m + 1) * 128]
        return out

    _big = [pack_km(inp['fWm'][i]).reshape(128, 512) for i in range(3)]
    _big += [pack_km(inp['fWp'][i]).reshape(128, 512) for i in range(3)]
    _big.append(pack_km(inp['fWs'][0:256]).reshape(128, 512))
    c['fW0big'] = np.concatenate(
        [inp['fW0'][perm63], inp['fWs'][256:][perm63]], 1)       # [63,512]
    c['fb0col'] = inp['fb0'].reshape(2, 128).T.copy()            # [128,2]
    for i in range(3):
        c[f'fbm{i}col'] = inp['fbm'][i].reshape(2, 128).T.copy()
        c[f'fbp{i}col'] = inp['fbp'][i].reshape(2, 128).T.copy()
    c['fbscol'] = inp['fbs'].reshape(2, 128).T.copy()

    # view head: fold Wfeat into Wview
    Wv = inp['Wview']
    Wv_d, Wv_emb, Wv_t, Wv_app = (Wv[256:283], Wv[283:331],
                                  Wv[331:344], Wv[344:383])
    Wfc = (inp['Wfeat'].astype(np.float64) @ Wv[0:256].astype(np.float64)
           ).astype(f32)
    out = np.zeros((128, 2, 128), f32)
    out[:, 0, :] = Wfc[0:128]
    out[:, 1, :] = Wfc[128:256]
    _big.append(out.reshape(128, 256))
    _fwbig = np.concatenate(_big, 1)                             # [128,3840]
    c['bveffcol'] = (inp['bfeat'].astype(np.float64)
                     @ Wv[0:256].astype(np.float64)
                     + inp['bview'].astype(np.float64)
                     ).astype(f32).reshape(-1, 1)
    perm39 = list(range(3, 39)) + [0, 1, 2]
    def _pad48(a):
        z = np.zeros((48, 128), f32)
        z[0:a.shape[0]] = a
        return z
    c['Wvpack'] = np.concatenate(
        [_pad48(Wv_app[perm39]), _pad48(Wv_d[0:3]), _pad48(Wv_d[3:27]),
         _pad48(Wv_emb), _pad48(Wv_t[0:1]), _pad48(Wv_t[1:13])],
        1).astype(f32)                                           # [48,768]
    c['fWbig'] = np.concatenate(
        [_fwbig,
         np.stack([inp['Wsig'][0:128, 0], inp['Wsig'][128:256, 0]], 1),
         inp['Wrgb']], 1).astype(f32)                            # [128,3845]
    c['brgbcol'] = inp['brgb'].reshape(-1, 1).copy()             # [3,1]
    c['iotacol'] = np.arange(100, dtype=f32).reshape(-1, 1)
    c['emb_table'] = inp['emb_table'].astype(f32)

    BP = [('pb0col', 1), ('pb1col', 1), ('pb2col', 1), ('fb0col', 2),
          ('fbm0col', 2), ('fbm1col', 2), ('fbm2col', 2), ('fbp0col', 2),
          ('fbp1col', 2), ('fbp2col', 2), ('fbscol', 2), ('bveffcol', 1),
          ('brgbcol', 1), ('iotacol', 1), ('cphasecol', 1)]
    bpack = np.zeros((128, 23), f32)
    _o = 0
    for _nm, _w in BP:
        _v = c.pop(_nm)
        bpack[0:_v.shape[0], _o:_o + _w] = _v
        _o += _w
    sip = np.zeros((128, 257), f32)
    sip[:, 0:129] = np.arange(129, dtype=f32) / 128.0
    sip[:, 129:257] = np.eye(128, dtype=f32)
    c['cpk'] = np.concatenate([c.pop('pwpack'), sip, bpack], 1)  # [128,537]
    etp = np.zeros((100, 560), f32)
    for rl in range(4):
        etp[rl, rl * 128:(rl + 1) * 128] = 1.0
    etp[:, 512:560] = c.pop('emb_table') if 'emb_table' in c else 0
    c['etpack'] = etp
    scalars = dict(pbo_f=float(inp['pbo'][0]), bsig_f=float(inp['bsig'][0]))
    return c, scalars


INPUT_SHAPES = {
    'rays': (R, 12),
    'geopack': (4, 312), 'cpk': (128, 537),
    'pW0my67': (67, 128),
    'fW0big': (63, 512), 'fWbig': (128, 3845),
    'Wvpack': (48, 768),
    'etpack': (100, 560),
}
F32R_WEIGHTS = {'fW0big', 'fWbig', 'Wvpack', 'etpack'}


# ---------------------------------------------------------------- bass build
def build_nc(pbo_f, bsig_f, stage=3, debug=False):
    nc = bacc.Bacc("TRN2", target_bir_lowering=False)
    D = {k: nc.dram_tensor(k, list(v), F32, kind="ExternalInput")
         for k, v in INPUT_SHAPES.items()}
    OUT = nc.dram_tensor("rgb_out", [R, 3], F32, kind="ExternalOutput")
    dbg = {}
    if debug:
        for nm, shp in [("d_sigc", (R, S)), ("d_zf", (R, S + 1)),
                        ("d_wc", (R, S)), ("d_sigf", (R, S)),
                        ("d_wf", (R, S)), ("d_hvray", (128, R)),
                        ("d_ec", (63, CN)), ("d_efa", (63, CN)),
                        ("d_efb", (39, CN)), ("d_h1", (128, 2 * TILE_N))]:
            dbg[nm] = nc.dram_tensor(nm, list(shp), F32, kind="ExternalOutput")
    with tile.TileContext(nc) as tc:
        _body(nc, tc, D, OUT, dbg, pbo_f, bsig_f, stage, debug)
    nc.compile()
    return nc


def _body(nc, tc, D, OUT, dbg, pbo_f, bsig_f, stage, debug):
    from contextlib import ExitStack
    ctx = ExitStack()
    wpool = ctx.enter_context(tc.tile_pool(name="w", bufs=1))
    per = ctx.enter_context(tc.tile_pool(name="per", bufs=1))
    pp2 = ctx.enter_context(tc.tile_pool(name="pp2", bufs=2))
    big = ctx.enter_context(tc.tile_pool(name="big", bufs=2))
    hp = ctx.enter_context(tc.tile_pool(name="h", bufs=3))
    dram = ctx.enter_context(tc.tile_pool(name="dr", bufs=2, space="DRAM"))
    psA = ctx.enter_context(tc.tile_pool(name="psA", bufs=4, space="PSUM"))
    psS = ctx.enter_context(tc.tile_pool(name="psS", bufs=1, space="PSUM"))
    psR = ctx.enter_context(tc.tile_pool(name="psR", bufs=1, space="PSUM"))
    psC = ctx.enter_context(tc.tile_pool(name="psC", bufs=1, space="PSUM"))

    # load in use-order: rays + per-ray consts + coarse weights first, so
    # the per-ray phase and coarse chunk 0 are not gated on fine weights
    EARLY = ['cpk', 'pW0my67', 'geopack', 'etpack', 'Wvpack']
    rays = wpool.tile([R, 12], F32, tag="w_rays")
    nc.sync.dma_start(rays[:], D['rays'][:])
    W = {}
    order = EARLY + [k for k in D if k not in EARLY and k != 'rays']
    _dmaeng = [nc.sync]
    for _i, k in enumerate(order):
        t = D[k]
        dt = F32R if k in F32R_WEIGHTS else F32
        tl = wpool.tile(list(t.shape), dt, tag="w_" + k)
        _dmaeng[_i % len(_dmaeng)].dma_start(
            tl[:], t[:].bitcast(F32R) if dt == F32R else t[:])
        W[k] = tl
    ident = W['cpk'][:, 386:514]
    _names = ['fWm0', 'fWm1', 'fWm2', 'fWp0', 'fWp1', 'fWp2', 'fWs_h']
    for _i, _nm in enumerate(_names):
        W[_nm] = W['fWbig'][:, _i * 512:(_i + 1) * 512].rearrange(
            "p (s m) -> p s m", s=4)
    W['Wfc'] = W['fWbig'][:, 3584:3840].rearrange("p (s m) -> p s m", s=2)
    W['fW0my'] = W['fW0big'][:, 0:256]
    W['fWs_e'] = W['fW0big'][:, 256:512]
    _wv = [('Wv_app', 39), ('Wv_d_lin', 3), ('Wv_d_sin', 24),
           ('Wv_emb', 48), ('Wv_t_lin', 1), ('Wv_t_sin', 12)]
    for _i, (_nm, _rows) in enumerate(_wv):
        W[_nm] = W['Wvpack'][0:_rows, _i * 128:(_i + 1) * 128]
    W['sgrid'] = W['cpk'][:, 257:386]
    W['identity'] = W['cpk'][:, 386:514]
    W['Etile'] = W['etpack'][0:4, 0:512]
    W['emb_table'] = W['etpack'][0:100, 512:560]
    W['cA3selT'] = W['geopack'][0:3, 248:312]
    W['AdT'] = W['geopack'][0:4, 0:24]
    W['AtT'] = W['geopack'][0:2, 24:36]
    W['fA3T'] = W['geopack'][0:3, 36:142]
    W['fA4T'] = W['geopack'][0:4, 142:248]
    W['pW1'] = W['cpk'][:, 0:128]
    W['pW2'] = W['cpk'][:, 128:256]
    W['pWo'] = W['cpk'][:, 256:257]
    W['Wsig'] = W['fWbig'][:, 3840:3842]
    W['Wrgb'] = W['fWbig'][:, 3842:3845]
    _bp = [('pb0col', 1, 128), ('pb1col', 1, 128), ('pb2col', 1, 128),
           ('fb0col', 2, 128), ('fbm0col', 2, 128), ('fbm1col', 2, 128),
           ('fbm2col', 2, 128), ('fbp0col', 2, 128), ('fbp1col', 2, 128),
           ('fbp2col', 2, 128), ('fbscol', 2, 128), ('bveffcol', 1, 128),
           ('brgbcol', 1, 3), ('iotacol', 1, 100), ('cphasecol', 1, 64)]
    _o = 514
    for _nm, _w, _rows in _bp:
        W[_nm] = W['cpk'][0:_rows, _o:_o + _w]
        _o += _w

    # ---------------- phase 0: per-ray prep (ray-major layout)
    nearc = per.tile([R, 1], F32)
    nc.vector.tensor_scalar(nearc[:], rays[:, 6:7], 1e-8, None, op0=OP.max)
    spanc = per.tile([R, 1], F32)
    nc.vector.tensor_tensor(spanc[:], rays[:, 7:8], nearc[:], op=OP.subtract)

    dsq = per.tile([R, 3], F32)
    nc.vector.tensor_tensor(dsq[:], rays[:, 3:6], rays[:, 3:6], op=OP.mult)
    ssum = per.tile([R, 1], F32)
    nc.vector.reduce_sum(ssum[:], dsq[:], axis=mybir.AxisListType.X)
    norm = per.tile([R, 1], F32)
    nc.scalar.activation(norm[:], ssum[:], AF.Sqrt)
    for it in range(2):
        t1 = per.tile([R, 1], F32, tag="nwt")
        nc.vector.reciprocal(t1[:], norm[:])
        nc.vector.scalar_tensor_tensor(t1[:], ssum[:], 1.0, t1[:],
                                       op0=OP.mult, op1=OP.mult)
        nc.vector.tensor_tensor(t1[:], t1[:], norm[:], op=OP.add)
        nc.vector.tensor_scalar(norm[:], t1[:], 0.5, None, op0=OP.mult)
    invn = per.tile([R, 1], F32)
    nc.vector.reciprocal(invn[:], norm[:])

    # bundle: 0:3 oc, 3 ones | 4:7 dc | 8:11 o, 11 ones | 12:15 dir |
    #         16:19 viewdir, 19 ones | 20 t, 21 ones | 22 embid
    bundle = per.tile([R, 28], F32)
    nc.gpsimd.memset(bundle[:], 0.0)
    nc.vector.scalar_tensor_tensor(bundle[:, 0:3], rays[:, 3:6], nearc[:],
                                   rays[:, 0:3], op0=OP.mult, op1=OP.add)
    nc.vector.memset(bundle[:, 3:4], 1.0)
    nc.vector.tensor_scalar(bundle[:, 4:7], rays[:, 3:6], spanc[:], None,
                            op0=OP.mult)
    nc.vector.tensor_copy(bundle[:, 8:11], rays[:, 0:3])
    nc.vector.memset(bundle[:, 11:12], 1.0)
    nc.vector.tensor_copy(bundle[:, 12:15], rays[:, 3:6])
    nc.vector.tensor_scalar(bundle[:, 16:19], rays[:, 3:6], invn[:], None,
                            op0=OP.mult)
    nc.vector.memset(bundle[:, 19:20], 1.0)
    nc.vector.tensor_copy(bundle[:, 20:21], rays[:, 8:9])
    nc.vector.memset(bundle[:, 21:22], 1.0)
    nc.vector.tensor_copy(bundle[:, 22:23], rays[:, 9:10])

    def transp(col):
        p = psC.tile([4, 128], F32, tag="pmc")
        nc.tensor.transpose(p[:], bundle[:, col:col + 4], ident[:])
        sb = per.tile([4, 128], F32, tag="tp%d" % col)
        nc.scalar.copy(sb[:], p[:])
        return sb

    ocT = transp(0)      # [ocT;ones]
    dcT = transp(4)      # [dcT;..]
    oT = transp(8)       # [oT;ones]
    dirT = transp(12)
    vdT = transp(16)     # [viewdirT;ones]
    tT = transp(20)      # [t;ones;embid]
    eiT = transp(22)     # row0 = embid (base 0 for partition_broadcast)

    def mm_copy(lhsT, rhs, shape, nm, dst_dtype=F32):
        p = psC.tile(shape, F32, tag="pmc")
        nc.tensor.matmul(p[:], lhsT, rhs, start=True, stop=True)
        sb = per.tile(shape, dst_dtype, tag="mc_" + nm)
        nc.scalar.copy(sb[:], p[:])
        return sb

    Bf = mm_copy(W['fA3T'][:], dirT[0:3, :], [106, 128], "Bf")
    Cf = mm_copy(W['fA4T'][:], oT[:], [106, 128], "Cf")

    def rangered_v(ap, shape, tag):
        sc = per.tile(shape, F32, tag=tag)
        nc.vector.tensor_scalar(sc[:], ap, float(INV2PI), float(MAGIC),
                                op0=OP.mult, op1=OP.add)
        nc.vector.tensor_scalar(sc[:], sc[:], float(MAGIC), None,
                                op0=OP.subtract)
        nc.vector.scalar_tensor_tensor(ap, sc[:], -float(C1), ap,
                                       op0=OP.mult, op1=OP.add)
        nc.vector.scalar_tensor_tensor(ap, sc[:], -float(C2), ap,
                                       op0=OP.mult, op1=OP.add)

    # per-ray view features
    argd = mm_copy(W['AdT'][:], vdT[:], [24, 128], 'argd')
    rangered_v(argd[:], [24, 128], "rrd")
    sind = per.tile([24, 128], F32R)
    nc.scalar.activation(sind[:], argd[:], AF.Sin)
    vd_r = per.tile([4, 128], F32R)
    nc.vector.tensor_copy(vd_r[:], vdT[:])

    argt = mm_copy(W['AtT'][:], tT[0:2, :], [12, 128], 'argt')
    rangered_v(argt[:], [12, 128], "rrt")
    sint = per.tile([12, 128], F32R)
    nc.scalar.activation(sint[:], argt[:], AF.Sin)
    t_r = per.tile([4, 128], F32R)
    nc.vector.tensor_copy(t_r[:], tT[:])

    embBC = per.tile([100, 128], F32)
    nc.gpsimd.partition_broadcast(embBC[:], eiT[0:1, :], channels=100)
    onehot = per.tile([100, 128], F32R)
    nc.vector.tensor_scalar(onehot[:], embBC[:], W['iotacol'][:], None,
                            op0=OP.is_equal)
    embT = mm_copy(W['emb_table'][:], onehot[:], [48, 128], 'embT', dst_dtype=F32R)

    phv = psC.tile([128, 128], F32, tag="pmc")
    nc.tensor.matmul(phv[:], W['Wv_d_lin'][:], vd_r[0:3, :],
                     start=True, stop=False)
    nc.tensor.matmul(phv[:], W['Wv_d_sin'][:], sind[:], start=False, stop=False)
    nc.tensor.matmul(phv[:], W['Wv_emb'][:], embT[:], start=False, stop=False)
    nc.tensor.matmul(phv[:], W['Wv_t_lin'][:], t_r[0:1, :],
                     start=False, stop=False)
    nc.tensor.matmul(phv[:], W['Wv_t_sin'][:], sint[:], start=False, stop=True)
    hvray = per.tile([128, 128], F32)
    nc.vector.tensor_scalar(hvray[:], phv[:], W['bveffcol'][:], None,
                            op0=OP.add)
    if debug:
        nc.sync.dma_start(dbg["d_hvray"][:], hvray[:])
    phvT = psC.tile([128, 128], F32, tag="pmc")
    nc.tensor.transpose(phvT[:], hvray[:], ident[:])
    hvrayT = per.tile([128, 128], F32R)
    nc.scalar.copy(hvrayT[:], phvT[:])
    hvb = dram.tile([128, 128], F32R, tag="hvb")
    nc.sync.dma_start(hvb[:], hvrayT[:])
    hvre = wpool.tile([4, 32, 128], F32R, tag="hvre")
    nc.sync.dma_start(hvre[:], hvb[:].rearrange("(t rl) m -> rl t m", rl=4))

    # coarse z edges
    zc = per.tile([R, S + 1], F32)
    nc.vector.tensor_scalar(zc[:], W['sgrid'][:], spanc[:], None, op0=OP.mult)
    nc.vector.tensor_scalar(zc[:], zc[:], nearc[:], None, op0=OP.add)
    midc = per.tile([R, S], F32)
    nc.vector.tensor_tensor(midc[:], zc[:, 0:S], zc[:, 1:S + 1], op=OP.add)
    nc.vector.tensor_scalar(midc[:], midc[:], 0.5, None, op0=OP.mult)

    # ======================= COARSE PASS =======================
    sigcT = per.tile([R, S], F32, tag="sigcT")

    def coarse_chunk(ci):
        r0 = ci * CHUNK_RAYS
        mbc = dram.tile([CHUNK_RAYS, S], F32, tag="midb")
        nc.sync.dma_start(mbc[:], midc[r0:r0 + CHUNK_RAYS, :])
        mfc = pp2.tile([1, CN], F32, tag="flat", bufs=1)
        nc.sync.dma_start(mfc[:],
                          mbc[:].rearrange("p f -> (p f)").unsqueeze(0))

        sb_ = dram.tile([1, CN], F32, tag="sigb")
        sigflat = pp2.tile([1, CN], F32, tag="sigflat", bufs=1)
        for tp in range(NTILE // 2):
            tpair = (2 * tp, 2 * tp + 1)
            colsof = {t: slice(t * TILE_N, (t + 1) * TILE_N) for t in tpair}
            xyzcs, ects = {}, {}
            for t in tpair:
                cols = colsof[t]
                r4 = slice(r0 + 4 * t, r0 + 4 * t + 4)
                xyzct = pp2.tile([3, TILE_N], F32, tag="xyzc", bufs=4,
                                 name="xyzc%d" % t)
                nc.gpsimd.partition_broadcast(xyzct[:], mfc[0:1, cols],
                                              channels=3)
                d3 = dirT[0:3, r4].unsqueeze(2).broadcast_to([3, 4, S])
                o3 = oT[0:3, r4].unsqueeze(2).broadcast_to([3, 4, S])
                x3 = xyzct[:].rearrange("p (r s) -> p r s", r=4)
                nc.vector.tensor_tensor(x3, x3, d3, op=OP.mult)
                nc.vector.tensor_tensor(x3, x3, o3, op=OP.add)
                xyzcs[t] = xyzct

                pa = psA.tile([64, TILE_N], F32, tag="mmps",
                              name="pa%d" % t)
                nc.tensor.matmul(pa[:], W['cA3selT'][:], xyzct[:],
                                 start=True, stop=True)
                ect = big.tile([67, TILE_N], F32, tag="ect", bufs=2,
                               name="ect%d" % t)
                nc.vector.tensor_scalar(ect[0:64, :], pa[:],
                                        W['cphasecol'][:], None, op0=OP.add)
                nc.scalar.copy(ect[64:67, :], xyzct[:])
                sct = big.tile([60, TILE_N], F32, tag="sct", bufs=1,
                               name="sct%d" % t)
                nc.gpsimd.tensor_scalar(sct[:], ect[0:60, :], float(INV2PI),
                                        float(MAGIC), op0=OP.mult, op1=OP.add)
                nc.gpsimd.tensor_scalar(sct[:], sct[:], float(MAGIC), None,
                                        op0=OP.subtract)
                nc.vector.scalar_tensor_tensor(
                    ect[0:60, :], sct[:], -float(np.float32(2.0 * np.pi)),
                    ect[0:60, :], op0=OP.mult, op1=OP.add)
                nc.scalar.activation(ect[0:60, :], ect[0:60, :], AF.Sin)
                ects[t] = ect

            p1s, h1s, p2s, h2s, p3s, h3s = {}, {}, {}, {}, {}, {}
            for t in tpair:
                p1 = psA.tile([128, TILE_N], F32, tag="mmps",
                              name="p1_%d" % t)
                nc.tensor.matmul(p1[:], W['pW0my67'][:], ects[t][:],
                                 start=True, stop=True)
                p1s[t] = p1
            for t in tpair:
                h1 = hp.tile([128, TILE_N], F32, tag="ch", bufs=2,
                             name="h1_%d" % t)
                nc.scalar.activation(h1[:], p1s[t][:], AF.Relu,
                                     bias=W['pb0col'][:])
                h1s[t] = h1
            for t in tpair:
                p2 = psA.tile([128, TILE_N], F32, tag="mmps",
                              name="p2_%d" % t)
                nc.tensor.matmul(p2[:], W['pW1'][:], h1s[t][:],
                                 start=True, stop=True)
                p2s[t] = p2
            for t in tpair:
                h2 = hp.tile([128, TILE_N], F32, tag="ch", bufs=2,
                             name="h2_%d" % t)
                nc.vector.tensor_scalar(h2[:], p2s[t][:], W['pb1col'][:],
                                        0.0, op0=OP.add, op1=OP.max)
                h2s[t] = h2
            for t in tpair:
                p3 = psA.tile([128, TILE_N], F32, tag="mmps",
                              name="p3_%d" % t)
                nc.tensor.matmul(p3[:], W['pW2'][:], h2s[t][:],
                                 start=True, stop=True)
                p3s[t] = p3
            for t in tpair:
                h3 = hp.tile([128, TILE_N], F32, tag="ch", bufs=2,
                             name="h3_%d" % t)
                nc.scalar.activation(h3[:], p3s[t][:], AF.Relu,
                                     bias=W['pb2col'][:])
                h3s[t] = h3
            for t in tpair:
                ps_ = psS.tile([1, TILE_N], F32, tag="sigps",
                               name="psg%d" % t)
                nc.tensor.matmul(ps_[:], W['pWo'][:], h3s[t][:],
                                 start=True, stop=True)
                nc.scalar.copy(sigflat[0:1, colsof[t]], ps_[:])
        nc.sync.dma_start(sb_[:], sigflat[:])
        nc.sync.dma_start(sigcT[r0:r0 + CHUNK_RAYS, :],
                          sb_[:].rearrange("a (p f) -> (a p) f", p=CHUNK_RAYS))

    # ======================= raw2weights helper =======================
    def raw2w(sigT_ap, z_lo, z_hi, norm_ap, bias_f, nrows, tag):
        """w_i = E_{i-1} - E_i with E = exp(-cumsum(relu(sig+b)*dz*norm))."""
        P = nrows
        dz = per.tile([P, S], F32, tag=tag + "dz")
        nc.vector.tensor_tensor(dz[:], z_hi, z_lo, op=OP.subtract)
        x = per.tile([P, S], F32, tag=tag + "x")
        nc.vector.tensor_scalar(x[:], sigT_ap, bias_f, 0.0,
                                op0=OP.add, op1=OP.max)
        nc.vector.tensor_tensor(x[:], x[:], dz[:], op=OP.mult)
        nc.vector.tensor_scalar(x[:], x[:], norm_ap, None, op0=OP.mult)
        Xc = per.tile([P, S], F32, tag=tag + "X")
        nc.vector.tensor_tensor_scan(Xc[:], x[:], x[:], 0.0,
                                     op0=OP.add, op1=OP.bypass)
        e = per.tile([P, S], F32, tag=tag + "e")
        nc.scalar.activation(e[:], Xc[:], AF.Exp, scale=-1.0)
        w = per.tile([P, S], F32, tag=tag + "w")
        nc.vector.tensor_scalar(w[:, 0:1], e[:, 0:1], -1.0, 1.0,
                                op0=OP.mult, op1=OP.add)
        nc.vector.tensor_tensor(w[:, 1:S], e[:, 0:S - 1], e[:, 1:S],
                                op=OP.subtract)
        return w, dz

    zf = per.tile([R, S + 1], F32)
    midf = per.tile([R, S], F32)
    PDF = {}

    def pdf_half(hi):
        h0 = hi * 64
        hs = slice(h0, h0 + 64)
        wc, dzc = raw2w(sigcT[hs, :], zc[hs, 0:S], zc[hs, 1:S + 1],
                        norm[hs, :], pbo_f, 64, "c%d" % hi)
        if debug:
            nc.sync.dma_start(dbg["d_wc"][hs, :], wc[:])
        Wt = per.tile([64, S], F32, tag="Wt%d" % hi)
        nc.vector.tensor_scalar(Wt[:], wc[:], 1e-5, None, op0=OP.add)
        Sx = per.tile([64, S], F32, tag="Sx%d" % hi)
        nc.vector.memset(Sx[:, 0:1], 0.0)
        nc.vector.tensor_tensor_scan(Sx[:, 1:S], Wt[:, 0:S - 1],
                                     Wt[:, 0:S - 1], 0.0,
                                     op0=OP.add, op1=OP.bypass)
        Tt = per.tile([64, 1], F32, tag="Tt%d" % hi)
        nc.vector.tensor_tensor(Tt[:], Sx[:, S - 1:S], Wt[:, S - 1:S],
                                op=OP.add)
        P2 = per.tile([64, S], F32, tag="P2%d" % hi)
        nc.vector.reciprocal(P2[:], Wt[:])
        nc.vector.tensor_tensor(P2[:], P2[:], dzc[:], op=OP.mult)
        Sn = Sx
        nc.vector.tensor_scalar(Sn[:], Sx[:], -1.0, None, op0=OP.mult)
        UT = per.tile([64, S + 1], F32, tag="UT%d" % hi)
        nc.vector.tensor_scalar(UT[:], W['sgrid'][0:64, :], Tt[:], None,
                                op0=OP.mult)
        PDF[hi] = dict(hs=hs, dzc=dzc, P2=P2, Sn=Sn, UT=UT)

    def pdf_js(hi, jlo, jhi):
        st = PDF[hi]
        hs = st['hs']
        for j in range(jlo, jhi):
            x_ = pp2.tile([64, S], F32, tag="pdfxD", name="x_")
            nc.vector.scalar_tensor_tensor(x_[:], st['Sn'][:],
                                           st['UT'][:, j:j + 1], st['P2'][:],
                                           op0=OP.add, op1=OP.mult)
            sc_ = pp2.tile([64, S], F32, tag="pdfxDs", name="sc_")
            nc.vector.scalar_tensor_tensor(sc_[:], x_[:], 0.0, st['dzc'][:],
                                           op0=OP.max, op1=OP.min,
                                           accum_out=zf[hs, j:j + 1])

    def pdf_final(hi):
        hs = PDF[hi]['hs']
        # endpoints are exact: u=0 -> z_near, u=1 -> z_far
        nc.vector.memset(zf[hs, 0:1], 0.0)
        nc.vector.tensor_tensor(zf[hs, S:S + 1], zc[hs, S:S + 1],
                                zc[hs, 0:1], op=OP.subtract)
        nc.vector.tensor_scalar(zf[hs, :], zf[hs, :], zc[hs, 0:1], None,
                                op0=OP.add)
        nc.vector.tensor_tensor(midf[hs, :], zf[hs, 0:S], zf[hs, 1:S + 1],
                                op=OP.add)
        nc.vector.tensor_scalar(midf[hs, :], midf[hs, :], 0.5, None,
                                op0=OP.mult)
        if debug:
            nc.sync.dma_start(dbg["d_zf"][hs, :], zf[hs, :])

    # drive coarse chunks with pdf-half-0 interleaved into chunks 4-6
    for ci in range(NCHUNK):
        coarse_chunk(ci)
        if ci == 4:
            pdf_half(0)
            pdf_js(0, 1, 43)
        elif ci == 5:
            pdf_js(0, 43, 86)
        elif ci == 6:
            pdf_js(0, 86, S)
    pdf_half(1)
    pdf_final(0)
    if debug:
        nc.sync.dma_start(dbg["d_sigc"][:], sigcT[:])
    if stage < 3:
        ctx.close()
        return

    # ======================= FINE PASS =======================
    rgbout = per.tile([128, 3], F32)
    SBD = {}     # ci -> sigma-flat DRAM tile
    RGBS = {}    # ci -> rgb pre-act [3, CN]

    def raw2w_fine(cj):
        r0j = cj * CHUNK_RAYS
        P = CHUNK_RAYS
        sigch = pp2.tile([P, S], F32, tag="sigch")
        nc.sync.dma_start(sigch[:],
                          SBD[cj][:].rearrange("a (p f) -> (a p) f", p=P))
        zfc = pp2.tile([P, S + 1], F32, tag="zfc")
        nc.sync.dma_start(zfc[:], zf[r0j:r0j + P, :])
        normc = pp2.tile([P, 1], F32, tag="normc")
        nc.sync.dma_start(normc[:], norm[r0j:r0j + P, :])
        wfj, _ = raw2w(sigch[:], zfc[:, 0:S], zfc[:, 1:S + 1],
                       normc[:], bsig_f, P, "fr%d" % (cj % 2))
        if debug:
            nc.sync.dma_start(dbg["d_sigf"][r0j:r0j + P, :], sigch[:])
            nc.sync.dma_start(dbg["d_wf"][r0j:r0j + P, :], wfj[:])
        return wfj

    def composite(cj, wfj):
        """rgb_map rows = ones^T @ (sigmoid(rgbT) * wfT) via PE transposes."""
        r0j = cj * CHUNK_RAYS
        rgbS_t = RGBS[cj]
        pwt = psC.tile([128, CHUNK_RAYS], F32, tag="pmc")
        nc.tensor.transpose(pwt[:], wfj[:], ident[0:CHUNK_RAYS, 0:CHUNK_RAYS])
        wfT = per.tile([128, CHUNK_RAYS], F32, tag="wfT", bufs=2)
        nc.vector.tensor_copy(wfT[:], pwt[:])
        prT = psC.tile([128, 3 * CHUNK_RAYS], F32, tag="pmc")
        for rl in range(CHUNK_RAYS):
            nc.tensor.transpose(prT[:, 3 * rl:3 * rl + 3],
                                rgbS_t[0:3, rl * S:(rl + 1) * S],
                                ident[0:3, 0:3])
        rgbT = per.tile([128, 3 * CHUNK_RAYS], F32, tag="rgbT", bufs=2)
        nc.scalar.activation(rgbT[:], prT[:], AF.Sigmoid)
        wr = per.tile([128, 3 * CHUNK_RAYS], F32, tag="wr", bufs=2)
        nc.vector.tensor_tensor(
            wr[:].rearrange("p (r c) -> p r c", r=CHUNK_RAYS),
            rgbT[:].rearrange("p (r c) -> p r c", r=CHUNK_RAYS),
            wfT[:].unsqueeze(2).broadcast_to([128, CHUNK_RAYS, 3]),
            op=OP.mult)
        po = psS.tile([1, 3 * CHUNK_RAYS], F32, tag="po", bufs=1)
        nc.tensor.matmul(po[:], W['sgrid'][:, S:S + 1], wr[:],
                         start=True, stop=True)
        posb = per.tile([1, 3 * CHUNK_RAYS], F32, tag="posb", bufs=1)
        nc.vector.tensor_copy(posb[:], po[:])
        ob = dram.tile([1, 3 * CHUNK_RAYS], F32, tag="ob")
        nc.sync.dma_start(ob[:], posb[:])
        nc.sync.dma_start(rgbout[r0j:r0j + CHUNK_RAYS, :],
                          ob[:].rearrange("a (r c) -> (a r) c", r=CHUNK_RAYS))

    for ci in range(NCHUNK):
        # pdf half-1 spread across fine chunks 0-3 (coarse data all ready)
        if ci < 4:
            jr = [1, 33, 66, 99, S]
            pdf_js(1, jr[ci], jr[ci + 1])
        elif ci == 4:
            pdf_final(1)
        r0 = ci * CHUNK_RAYS
        mb = dram.tile([CHUNK_RAYS, S], F32, tag="midb")
        nc.sync.dma_start(mb[:], midf[r0:r0 + CHUNK_RAYS, :])
        mflat = pp2.tile([1, CN], F32, tag="flat", bufs=1)
        nc.sync.dma_start(mflat[:],
                          mb[:].rearrange("p f -> (p f)").unsqueeze(0))
        mBC = big.tile([106, CN], F32, tag="mbcrr", bufs=2)
        nc.gpsimd.partition_broadcast(mBC[:], mflat[:], channels=106)

        argf = big.tile([106, CN], F32, tag="arg")
        b3 = Bf[:, r0:r0 + CHUNK_RAYS].unsqueeze(2).broadcast_to(
            [106, CHUNK_RAYS, S])
        c3 = Cf[:, r0:r0 + CHUNK_RAYS].unsqueeze(2).broadcast_to(
            [106, CHUNK_RAYS, S])
        a3 = argf[:].rearrange("p (r s) -> p r s", r=CHUNK_RAYS)
        m3 = mBC[:].rearrange("p (r s) -> p r s", r=CHUNK_RAYS)
        nc.vector.tensor_tensor(a3, m3, b3, op=OP.mult)
        nc.gpsimd.tensor_tensor(a3, a3, c3, op=OP.add)
        sc = mBC
        TWOPI = float(np.float32(2.0 * np.pi))
        for lo, hi in ((0, 60), (64, 100)):
            nc.gpsimd.tensor_scalar(sc[lo:hi, :], argf[lo:hi, :], float(INV2PI),
                                    float(MAGIC), op0=OP.mult, op1=OP.add)
            nc.gpsimd.tensor_scalar(sc[lo:hi, :], sc[lo:hi, :], float(MAGIC),
                                    None, op0=OP.subtract)
            nc.vector.scalar_tensor_tensor(argf[lo:hi, :], sc[lo:hi, :],
                                           -TWOPI, argf[lo:hi, :],
                                           op0=OP.mult, op1=OP.add)
        efa = big.tile([63, CN], F32R, tag="efa")
        efb = big.tile([39, CN], F32R, tag="efb")
        nc.scalar.activation(efa[0:60, :], argf[0:60, :], AF.Sin)
        nc.scalar.activation(efb[0:36, :], argf[64:100, :], AF.Sin)
        nc.sync.dma_start(efa[60:63, :], argf[100:103, :].bitcast(F32R))
        nc.sync.dma_start(efb[36:39, :], argf[103:106, :].bitcast(F32R))
        if debug and ci == 0:
            nc.sync.dma_start(dbg["d_efa"][:], efa[:].bitcast(F32))
            nc.sync.dma_start(dbg["d_efb"][:], efb[:].bitcast(F32))

        rgbS = big.tile([3, CN], F32, tag="rgbS", bufs=2)
        RGBS[ci] = rgbS
        sb_ = dram.tile([1, CN], F32, tag="sigbf", bufs=3)
        SBD[ci] = sb_
        sigflat = pp2.tile([1, CN], F32, tag="sigflat", bufs=1)
        # relu engine per (layer, half): A=Activation, D=DVE, P=Pool
        if ci < 4:
            # pdf half-1 loads DVE here; lean on Act
            RELU_ENG = [('A', 'D'), ('A', 'A'), ('A', 'D'), ('A', 'D'),
                        ('A', 'D'), ('A', 'A'), ('A', 'D'), ('A', 'D')]
        else:
            RELU_ENG = [('A', 'D'), ('A', 'D'), ('A', 'D'), ('A', 'D'),
                        ('A', 'D'), ('A', 'A'), ('A', 'D'), ('A', 'D')]

        def relu_half(dst, src, bias_ap, eng):
            if eng == 'A':
                nc.scalar.activation(dst, src, AF.Relu, bias=bias_ap)
            elif eng == 'D':
                nc.vector.tensor_scalar(dst, src, bias_ap, 0.0,
                                        op0=OP.add, op1=OP.max)
            else:
                nc.gpsimd.tensor_scalar(dst, src, bias_ap, 0.0,
                                        op0=OP.add, op1=OP.max)

        # layer-interleaved over tile pairs: PE fills each layer's relu
        # latency of tile t with the other tile's matmuls
        for tp in range(NTILE // 2):
            tpair = (2 * tp, 2 * tp + 1)
            colsof = {t: slice(t * TILE_N, (t + 1) * TILE_N) for t in tpair}

            hcur = {}
            pm = {}
            for t in tpair:
                pm[t] = [psA.tile([128, TILE_N], F32, tag="mmps",
                                  name="pm%d_%d" % (t, _m)) for _m in range(2)]
                for m in range(2):
                    nc.tensor.matmul(pm[t][m][:],
                                     W['fW0my'][:, m * 128:(m + 1) * 128],
                                     efa[:, colsof[t]], start=True, stop=True)
            for t in tpair:
                hcur[t] = hp.tile([128, 2 * TILE_N], F32R, tag="fh",
                                  name="h%d" % t)
                relu_half(hcur[t][:, 0:TILE_N], pm[t][0][:],
                          W['fb0col'][:, 0:1], RELU_ENG[0][0])
                relu_half(hcur[t][:, TILE_N:], pm[t][1][:],
                          W['fb0col'][:, 1:2], RELU_ENG[0][1])
            if debug and ci == 0 and tp == 0:
                nc.sync.dma_start(dbg["d_h1"][:], hcur[0][:].bitcast(F32))

            def mid_layer_pair(wname, bname, li, skip=False):
                pmm = {}
                for t in tpair:
                    pmm[t] = [psA.tile([128, TILE_N], F32, tag="mmps",
                                       name="pmm%d_%d" % (t, _m))
                              for _m in range(2)]
                    for m in range(2):
                        nc.tensor.matmul(pmm[t][m][:], W[wname][:, m, :],
                                         hcur[t][:, 0:TILE_N],
                                         start=True, stop=False)
                        nc.tensor.matmul(pmm[t][m][:], W[wname][:, 2 + m, :],
                                         hcur[t][:, TILE_N:],
                                         start=False, stop=not skip)
                        if skip:
                            nc.tensor.matmul(
                                pmm[t][m][:],
                                W['fWs_e'][:, m * 128:(m + 1) * 128],
                                efa[:, colsof[t]], start=False, stop=True)
                for t in tpair:
                    hout = hp.tile([128, 2 * TILE_N], F32R, tag="fh",
                                   name="ho%d" % t)
                    relu_half(hout[:, 0:TILE_N], pmm[t][0][:],
                              W[bname][:, 0:1], RELU_ENG[li][0])
                    relu_half(hout[:, TILE_N:], pmm[t][1][:],
                              W[bname][:, 1:2], RELU_ENG[li][1])
                    hcur[t] = hout

            mid_layer_pair('fWm0', 'fbm0col', 1)
            mid_layer_pair('fWm1', 'fbm1col', 2)
            mid_layer_pair('fWm2', 'fbm2col', 3)
            mid_layer_pair('fWs_h', 'fbscol', 4, skip=True)
            mid_layer_pair('fWp0', 'fbp0col', 5)
            mid_layer_pair('fWp1', 'fbp1col', 6)
            mid_layer_pair('fWp2', 'fbp2col', 7)

            pvs = {}
            for t in tpair:
                h = hcur[t]
                cols = colsof[t]
                ps_ = psS.tile([1, TILE_N], F32, tag="sigps",
                               name="ps%d" % t)
                nc.tensor.matmul(ps_[:], W['Wsig'][:, 0:1], h[:, 0:TILE_N],
                                 start=True, stop=False)
                nc.tensor.matmul(ps_[:], W['Wsig'][:, 1:2], h[:, TILE_N:],
                                 start=False, stop=True)
                nc.scalar.copy(sigflat[0:1, cols], ps_[:])

                pv = psA.tile([128, TILE_N], F32, tag="mmps",
                              name="pv%d" % t)
                nc.tensor.matmul(pv[:], W['Wfc'][:, 0, :], h[:, 0:TILE_N],
                                 start=True, stop=False)
                nc.tensor.matmul(pv[:], W['Wfc'][:, 1, :], h[:, TILE_N:],
                                 start=False, stop=False)
                nc.tensor.matmul(pv[:], W['Wv_app'][:], efb[:, cols],
                                 start=False, stop=False)
                nc.tensor.matmul(pv[:], hvre[:, ci * NTILE + t, :],
                                 W['Etile'][:], start=False, stop=True)
                pvs[t] = pv
            for t in tpair:
                cols = colsof[t]
                hv = hp.tile([128, TILE_N], F32R, tag="fhv", bufs=1,
                             name="hv%d" % t)
                nc.vector.tensor_scalar(hv[:], pvs[t][:], 0.0, None,
                                        op0=OP.max)
                prgb = psR.tile([3, TILE_N], F32, tag="rgbps",
                                name="prgb%d" % t)
                nc.tensor.matmul(prgb[:], W['Wrgb'][:], hv[:],
                                 start=True, stop=True)
                nc.vector.tensor_scalar(rgbS[0:3, cols], prgb[:],
                                        W['brgbcol'][:], None, op0=OP.add)

        nc.sync.dma_start(sb_[:], sigflat[:])
        if ci % 2 == 1:
            wA = raw2w_fine(ci - 1)
            wB = raw2w_fine(ci)
            composite(ci - 1, wA)
            composite(ci, wB)

    nc.sync.dma_start(OUT[:], rgbout[:])
    ctx.close()


# ---------------------------------------------------------------- entry
_CACHE = {}


def kernel(**inputs):
    inp = {k: np.asarray(v) for k, v in inputs.items()}
    consts, scal = host_prep(inp)
    key = (BUILD_STAGE, DEBUG_OUT, scal['pbo_f'], scal['bsig_f'])
    if key not in _CACHE:
        _CACHE[key] = build_nc(scal['pbo_f'], scal['bsig_f'],
                               stage=BUILD_STAGE, debug=DEBUG_OUT)
    nc = _CACHE[key]
    rays = np.asarray(inp['rays'], np.float32)
    in_maps = []
    for core in range(NCORES):
        m = {k: np.ascontiguousarray(v, dtype=np.float32)
             for k, v in consts.items()}
        m['rays'] = np.ascontiguousarray(rays[core * R:(core + 1) * R])
        in_maps.append(m)
    res = run_bass_kernel_spmd(nc, in_maps, core_ids=list(range(NCORES)))
    globals()['_LAST_RESULTS'] = res
    return np.concatenate([r['rgb_out'] for r in res.results], 0)

